# revision 1
# baseline (speedup 1.0000x reference)
"""Trainium2 Bass kernel for LocalSelfAttention (sliding-window, causal).

Problem: val (S=4096, B=2, D=768); q/k/v projections then Longformer-style
banded causal attention, window = 256 lookback (keys j in [i-256, i]).

Sharding: 8 cores = batch (2) x sequence quarters (4). Each core handles
1024 queries of one batch element and receives a 256-row key/value halo
(recomputed locally from val rows; no inter-core communication).

Math simplifications (exact up to float rounding):
  - bk dropped: per-query additive constant q.bk cancels in softmax.
  - bv added on host at the end: sum_j p_j (v0_j + bv) = (PV)/Z + bv.
  - no max-subtraction in softmax: scores ~ N(0,1), |s| < ~8, exp is safe.
  - 1/sqrt(hd) folded into Wq/bq on host.

Device pipeline per core (fp32r matmuls for projections+QK, bf16 for PV):
  valT (768,1280) --[PE]--> qT (768,1024), kT (768,1280), v_aug (1280, 12*65)
  per (head, 256-query group): scoresT = kT.T-slices @ qT-slices (keys on
  partitions), exp on ACT, banded 0/1 mask multiply on DVE, PV matmul with a
  ones-column in v_aug producing row-sums, reciprocal + per-partition scale.
"""

import os
import numpy as np
import ml_dtypes

S, B, D = 4096, 2, 768
H, HD = 12, 64
W = 256
NCORES = 8
SQ = S // 4            # 1024 queries per core
SKV = SQ + W           # 1280 kv rows (halo)
NQT = SQ // 128        # 8 query tiles
NG = SQ // 256         # 4 query groups
ND = D // 128          # 6 feature tiles
NKVT = SKV // 128      # 10 kv tiles
VA = HD + 1            # 65: per-head v width incl. ones column
SCALE = 1.0 / np.sqrt(HD).astype(np.float32)  # 0.125

_CACHE = {}


def _masks_np(boundary: bool) -> np.ndarray:
    """(2, 128, 768) bf16. Layout per set: [kt0h0(128) kt1(256) kt2(256) kt3h1(128)].

    Partition = key-within-tile pp; free = query-within-group r.
    Set 0 is used for group 0 (zeroed kt0/kt1 on sequence-boundary cores),
    set 1 for groups 1..3.
    """
    pp = np.arange(128)[:, None]
    r256 = np.arange(256)[None, :]
    r128 = np.arange(128)[None, :]
    kt0 = r128 <= pp            # keys [g*256-256, g*256-128), queries [0,128)
    kt1 = r256 <= 128 + pp      # keys [g*256-128, g*256)
    kt2 = r256 >= pp            # keys [g*256, g*256+128)
    kt3 = r128 >= pp            # keys [g*256+128, g*256+256), queries [128,256)
    setB = np.concatenate([kt0, kt1, kt2, kt3], axis=1).astype(np.float32)
    setA = setB.copy()
    if boundary:
        setA[:, 0:384] = 0.0    # kt0+kt1 keys are before row 0 -> invalid
    m = np.stack([setA, setB])
    return np.ascontiguousarray(m.astype(ml_dtypes.bfloat16))


def _build_nc():
    import concourse.bacc as bacc
    import concourse.mybir as mybir
    from concourse.tile import TileContext

    f32 = mybir.dt.float32
    f32r = mybir.dt.float32r
    bf16 = mybir.dt.bfloat16
    AF = mybir.ActivationFunctionType

    nc = bacc.Bacc(trn_type="TRN2", debug=False, num_devices=NCORES)

    valT_d = nc.dram_tensor("valT", [D, SKV], f32r, kind="ExternalInput").ap()
    wq_d = nc.dram_tensor("wq", [D, D], f32r, kind="ExternalInput").ap()
    wk_d = nc.dram_tensor("wk", [D, D], f32r, kind="ExternalInput").ap()
    wv_d = nc.dram_tensor("wv", [D, D], f32r, kind="ExternalInput").ap()
    bq_d = nc.dram_tensor("bq", [D, 1], f32, kind="ExternalInput").ap()
    masks_d = nc.dram_tensor("masks", [2, 128, 768], bf16, kind="ExternalInput").ap()
    out_d = nc.dram_tensor("out", [SQ, H * VA], f32, kind="ExternalOutput").ap()

    with TileContext(nc) as tc:
        with tc.tile_pool(name="persist", bufs=1) as pp:
            qT = [pp.tile([128, SQ], bf16, name=f"qT{m}", tag=f"qT{m}") for m in range(ND)]
            kT = [pp.tile([128, SKV], bf16, name=f"kT{m}", tag=f"kT{m}") for m in range(ND)]
            vaug = [pp.tile([128, H * VA], bf16, name=f"vaug{t}", tag=f"vaug{t}") for t in range(NKVT)]
            outsb = [pp.tile([128, H * VA], f32, name=f"outsb{q}", tag=f"outsb{q}") for q in range(NQT)]
            bqt = [pp.tile([128, 1], f32, name=f"bqt{m}", tag=f"bqt{m}") for m in range(ND)]
            maskt = [pp.tile([128, 768], bf16, name=f"maskt{i}", tag=f"maskt{i}") for i in range(2)]

            for m in range(ND):
                nc.scalar.dma_start(bqt[m][:], bq_d[m * 128:(m + 1) * 128, :])
            for i in range(2):
                nc.scalar.dma_start(maskt[i][:], masks_d[i])
            for t in range(NKVT):
                ones_col = vaug[t][:].rearrange("p (h c) -> p h c", c=VA)[:, :, HD:VA]
                nc.vector.memset(ones_col, 1.0)

            # ---------------- Phase A: load + projections ----------------
            with tc.tile_pool(name="stage", bufs=1) as sp, \
                 tc.tile_pool(name="pps", bufs=4, space="PSUM") as pps:
                # interleave + spread input DMAs across engine queues so the
                # first projection matmul's inputs land within a few us
                valT_t, wq_t, wk_t, wv_t = [], [], [], []
                for k in range(ND):
                    tw = sp.tile([128, D], f32r, name=f"wqt{k}", tag=f"wqt{k}")
                    nc.gpsimd.dma_start(tw[:], wq_d[k * 128:(k + 1) * 128, :])
                    wq_t.append(tw)
                    tv = sp.tile([128, SKV], f32r, name=f"valTt{k}", tag=f"valTt{k}")
                    nc.sync.dma_start(tv[:], valT_d[k * 128:(k + 1) * 128, :])
                    valT_t.append(tv)
                for k in range(ND):
                    t = sp.tile([128, D], f32r, name=f"wkt{k}", tag=f"wkt{k}")
                    nc.scalar.dma_start(t[:], wk_d[k * 128:(k + 1) * 128, :])
                    wk_t.append(t)
                for k in range(ND):
                    t = sp.tile([128, D], f32r, name=f"wvt{k}", tag=f"wvt{k}")
                    nc.gpsimd.dma_start(t[:], wv_d[k * 128:(k + 1) * 128, :])
                    wv_t.append(t)

                # qT[m][:, ch] = sum_k Wq[k,m].T @ valT[k, local cols]  (+bq)
                for m in range(ND):
                    for ch in range(2):
                        ps = pps.tile([128, 512], f32, name="psq", tag="psproj")
                        for k in range(ND):
                            nc.tensor.matmul(
                                ps[:],
                                wq_t[k][:, m * 128:(m + 1) * 128],
                                valT_t[k][:, W + ch * 512:W + (ch + 1) * 512],
                                start=(k == 0), stop=(k == ND - 1))
                        nc.scalar.activation(
                            qT[m][:, ch * 512:(ch + 1) * 512], ps[:],
                            AF.Identity, bias=bqt[m][:], scale=1.0)

                for m in range(ND):
                    for lo, hi in ((0, 512), (512, 1024), (1024, 1280)):
                        ps = pps.tile([128, hi - lo], f32, name="psk", tag="psproj")
                        for k in range(ND):
                            nc.tensor.matmul(
                                ps[:],
                                wk_t[k][:, m * 128:(m + 1) * 128],
                                valT_t[k][:, lo:hi],
                                start=(k == 0), stop=(k == ND - 1))
                        nc.vector.tensor_copy(kT[m][:, lo:hi], ps[:])

                # v natural: tokens on partitions; scatter per-head into vaug
                for t in range(NKVT):
                    for lo, hi, h0 in ((0, 512, 0), (512, 768, 8)):
                        ps = pps.tile([128, hi - lo], f32, name="psv", tag="psproj")
                        for k in range(ND):
                            nc.tensor.matmul(
                                ps[:],
                                valT_t[k][:, t * 128:(t + 1) * 128],
                                wv_t[k][:, lo:hi],
                                start=(k == 0), stop=(k == ND - 1))
                        nh = (hi - lo) // HD
                        src = ps[:].rearrange("p (h c) -> p h c", c=HD)
                        dst = vaug[t][:].rearrange("p (h c) -> p h c", c=VA)[:, h0:h0 + nh, 0:HD]
                        nc.vector.tensor_copy(dst, src)

            # ---------------- Phase B: banded attention ----------------
            kstage = int(os.environ.get("KSTAGE", "3"))
            if kstage >= 2:
              with tc.tile_pool(name="probsp", bufs=4) as prp, \
                 tc.tile_pool(name="scps", bufs=3, space="PSUM") as scp, \
                 tc.tile_pool(name="pvps", bufs=2, space="PSUM") as pvp, \
                 tc.tile_pool(name="smallp", bufs=8) as smp:
                iters = [(h, g) for g in range(NG) for h in range(H)]
                LAG = 2
                probs_ring = {}
                for i in range(len(iters) + LAG):
                    if i < len(iters):
                        h, g = iters[i]
                        mh, ph = h // 2, (h % 2) * 64
                        ps_s = scp.tile([128, 1024], f32, name="pss", tag="scores")
                        for kt in range(4):
                            ka = 2 * g + kt
                            nc.tensor.matmul(
                                ps_s[:, kt * 256:(kt + 1) * 256],
                                kT[mh][ph:ph + 64, ka * 128:(ka + 1) * 128],
                                qT[mh][ph:ph + 64, g * 256:(g + 1) * 256],
                                start=True, stop=True)
                        probs = prp.tile([128, 1024], bf16, name="probs", tag="probs")
                        nc.scalar.activation(probs[:], ps_s[:], AF.Exp)
                        mt = maskt[0] if g == 0 else maskt[1]
                        nc.vector.tensor_mul(probs[:, 0:128], probs[:, 0:128], mt[:, 0:128])
                        nc.vector.tensor_mul(probs[:, 256:768], probs[:, 256:768], mt[:, 128:640])
                        nc.vector.tensor_mul(probs[:, 896:1024], probs[:, 896:1024], mt[:, 640:768])
                        probs_ring[i] = probs
                    if kstage >= 3 and i >= LAG:
                        h, g = iters[i - LAG]
                        probs = probs_ring.pop(i - LAG)
                        for qs in range(2):
                            kts = (0, 1, 2) if qs == 0 else (1, 2, 3)
                            ps_o = pvp.tile([128, VA], f32, name="pso", tag="pv")
                            for n, kt in enumerate(kts):
                                nc.tensor.matmul(
                                    ps_o[:],
                                    probs[:, kt * 256 + qs * 128: kt * 256 + qs * 128 + 128],
                                    vaug[2 * g + kt][:, h * VA:(h + 1) * VA],
                                    start=(n == 0), stop=(n == len(kts) - 1))
                            qt = g * 2 + qs
                            nc.vector.tensor_copy(
                                outsb[qt][:, h * VA:(h + 1) * VA], ps_o[:])
                        if h == H - 1:
                            for qt in (2 * g, 2 * g + 1):
                                nc.sync.dma_start(
                                    out_d[qt * 128:(qt + 1) * 128, :], outsb[qt][:])

            if kstage < 3:
                for q in range(NQT):
                    nc.vector.memset(outsb[q][:], 0.0)
                for q in range(NQT):
                    nc.sync.dma_start(out_d[q * 128:(q + 1) * 128, :], outsb[q][:])
    nc.compile()
    return nc


def _get_nc():
    if "nc" not in _CACHE:
        _CACHE["nc"] = _build_nc()
    return _CACHE["nc"]


def _install_ntff_hook():
    """Provide antenv.axon_hooks (absent in this image) so bass_utils can
    NTFF-profile under axon, using trn_agent_boot's ctypes hook builder."""
    import sys
    import types
    try:
        from antenv.axon_hooks import get_axon_ntff_profile_hook  # noqa: F401
        return
    except ImportError:
        pass
    try:
        import antenv
        from trn_agent_boot.trn_boot import _ntff_profile_via_ctypes
        hook = _ntff_profile_via_ctypes("/opt/axon/libaxon_pjrt.so")
        mod = types.ModuleType("antenv.axon_hooks")
        mod.get_axon_ntff_profile_hook = lambda: hook
        mod.set_axon_ntff_profile_hook = lambda h: None
        sys.modules["antenv.axon_hooks"] = mod
        antenv.axon_hooks = mod
    except Exception as e:  # profiling is best-effort
        print(f"ntff hook install failed: {e}")


def kernel(val, Wq, bq, Wk, bk, Wv, bv):
    from concourse.bass_utils import run_bass_kernel_spmd

    val = np.asarray(val, dtype=np.float32)
    Wq = np.asarray(Wq, dtype=np.float32)
    bq = np.asarray(bq, dtype=np.float32)
    Wk = np.asarray(Wk, dtype=np.float32)
    Wv = np.asarray(Wv, dtype=np.float32)
    bv = np.asarray(bv, dtype=np.float32)

    wq_s = np.ascontiguousarray(Wq * SCALE)
    bq_s = np.ascontiguousarray((bq * SCALE).reshape(D, 1))
    wk_c = np.ascontiguousarray(Wk)
    wv_c = np.ascontiguousarray(Wv)

    in_maps = []
    for c in range(NCORES):
        b, qd = divmod(c, 4)
        lo = qd * SQ - W
        hi = qd * SQ + SQ
        vs = val[max(lo, 0):hi, b, :]
        if lo < 0:
            vs = np.concatenate([np.zeros((-lo, D), np.float32), vs], axis=0)
        in_maps.append({
            "valT": np.ascontiguousarray(vs.T),
            "wq": wq_s, "wk": wk_c, "wv": wv_c, "bq": bq_s,
            "masks": _masks_np(boundary=(qd == 0)),
        })

    nc = _get_nc()
    trace = os.environ.get("BASS_KERNEL_TRACE", "0") == "1"
    kwargs = {}
    if trace:
        _install_ntff_hook()
        kwargs = dict(trace=True, tmpdir=os.environ.get("BASS_KERNEL_TRACE_DIR") or None)
    res = run_bass_kernel_spmd(nc, in_maps, list(range(NCORES)), **kwargs)
    _CACHE["last_result"] = res

    out = np.empty((S, B, D), np.float32)
    for c in range(NCORES):
        b, qd = divmod(c, 4)
        raw = res.results[c]["out"].reshape(SQ, H, VA)
        out[qd * SQ:(qd + 1) * SQ, b, :] = (
            raw[:, :, 0:HD] / raw[:, :, HD:VA]).reshape(SQ, D)
    out += bv
    return out



# revision 7
# speedup vs baseline: 1.5251x; 1.5251x over previous
"""Trainium2 Bass kernel for LocalSelfAttention (sliding-window, causal).

Problem: val (S=4096, B=2, D=768); q/k/v projections then Longformer-style
banded causal attention, window = 256 lookback (keys j in [i-256, i]).

Sharding: 8 cores = batch (2) x sequence quarters (4). Each core handles
1024 queries of one batch element and receives a 256-row key/value halo
(recomputed locally from val rows; no inter-core communication).

Math simplifications (exact up to float rounding):
  - bk dropped: per-query additive constant q.bk cancels in softmax.
  - bv added on host at the end: sum_j p_j (v0_j + bv) = (PV)/Z + bv.
  - no max-subtraction in softmax: scores ~ N(0,1), |s| < ~8, exp is safe.
  - 1/sqrt(hd) folded into Wq/bq on host.

v2: single interleaved phase. All inputs bf16 (FWL weight loads, half the
DMA). Projection psum tiles, attention score units and PV units are woven
into one emission stream so every engine (PE / ACT exp / DVE casts+masks)
stays busy and psum ring recycling latency hides under other PE work.

Scores psum column layout per (head, 256-query group) is [kt1 kt0 kt3 kt2]
(256 cols each, kt = key tile of 128; kt0/kt1 = the 256 keys before the
group, kt2/kt3 = the group's own keys). This makes the masked + dead region
one contiguous 768-col block (cols 128:896), handled by a single DVE
multiply. PV accumulates a head-pair into one [128, 260] psum bank:
cols = qs*130 + hh*65 + (64 values + 1 ones-column row-sum).
"""

import os
import numpy as np
import ml_dtypes

S, B, D = 4096, 2, 768
H, HD = 12, 64
W = 256
NCORES = 8
SQ = S // 4            # 1024 queries per core
SKV = SQ + W           # 1280 kv rows (halo)
NQT = SQ // 128        # 8 query tiles
NG = SQ // 256         # 4 query groups
ND = D // 128          # 6 feature tiles (also head pairs)
NKVT = SKV // 128      # 10 kv tiles
VA = HD + 1            # 65: per-head v width incl. ones column
SCALE = 1.0 / np.sqrt(HD).astype(np.float32)  # 0.125

_CACHE = {}


def _masks_np(boundary: bool) -> np.ndarray:
    """(2, 128, 1024) bf16 multiplicative masks for the [kt1 kt0 kt3 kt2]
    score layout. Partition = key-within-tile p; free = 4 blocks of 256
    query rows r (r in 0..255 within the group).

    Valid patterns (c = r mod 128):
      kt1 block (cols 0:256):    r<128 all-valid; r>=128 tri c<=p
      kt0 block (cols 256:512):  r<128 tri c<=p;  r>=128 dead
      kt3 block (cols 512:768):  r<128 dead;      r>=128 tri c>=p
      kt2 block (cols 768:1024): r<128 tri c>=p;  r>=128 all-valid
    Set 0 is used for group 0 (multiplied over the full 1024 cols; on
    sequence-boundary cores kt1+kt0 are entirely invalid), set 1 for
    groups 1..3 (multiplied over cols 128:896 only).
    """
    p = np.arange(128)[:, None]
    c = np.arange(128)[None, :]
    triL = (c <= p).astype(np.float32)
    triU = (c >= p).astype(np.float32)
    setB = np.ones((128, 1024), np.float32)
    setB[:, 128:256] = triL
    setB[:, 256:384] = triL
    setB[:, 384:640] = 0.0
    setB[:, 640:768] = triU
    setB[:, 768:896] = triU
    setA = setB.copy()
    if boundary:
        setA[:, 0:512] = 0.0    # kt1+kt0 keys are before row 0 -> invalid
    m = np.stack([setA, setB])
    return np.ascontiguousarray(m.astype(ml_dtypes.bfloat16))


def _build_nc():
    import concourse.bacc as bacc
    import concourse.mybir as mybir
    from concourse.tile import TileContext

    f32 = mybir.dt.float32
    bf16 = mybir.dt.bfloat16
    AF = mybir.ActivationFunctionType

    nc = bacc.Bacc(trn_type="TRN2", debug=False, num_devices=NCORES)

    valT_d = nc.dram_tensor("valT", [D, SKV], bf16, kind="ExternalInput").ap()
    wq_d = nc.dram_tensor("wq", [D, D], bf16, kind="ExternalInput").ap()
    wk_d = nc.dram_tensor("wk", [D, D], bf16, kind="ExternalInput").ap()
    wv_d = nc.dram_tensor("wv", [D, D], bf16, kind="ExternalInput").ap()
    bq_d = nc.dram_tensor("bq", [D, 1], f32, kind="ExternalInput").ap()
    masks_d = nc.dram_tensor("masks", [2, 128, 1024], bf16, kind="ExternalInput").ap()
    out_d = nc.dram_tensor("out", [ND * NQT * 128, 2 * VA], f32, kind="ExternalOutput").ap()

    with TileContext(nc) as tc:
        with tc.tile_pool(name="persist", bufs=1) as pp, \
             tc.tile_pool(name="projps", bufs=2, space="PSUM") as projps, \
             tc.tile_pool(name="scps", bufs=2, space="PSUM") as scps, \
             tc.tile_pool(name="pvps", bufs=2, space="PSUM") as pvps, \
             tc.tile_pool(name="probsp", bufs=6) as prp, \
             tc.tile_pool(name="outp", bufs=6) as outp:

            qT = [pp.tile([128, SQ], bf16, name=f"qT{m}", tag=f"qT{m}") for m in range(ND)]
            kT = [pp.tile([128, SKV], bf16, name=f"kT{m}", tag=f"kT{m}") for m in range(ND)]
            vaug = [pp.tile([128, H * VA], bf16, name=f"vaug{t}", tag=f"vaug{t}") for t in range(NKVT)]
            bqt = [pp.tile([128, 1], f32, name=f"bqt{m}", tag=f"bqt{m}") for m in range(ND)]
            maskt = [pp.tile([128, 1024], bf16, name=f"maskt{i}", tag=f"maskt{i}") for i in range(2)]
            valT_t = [pp.tile([128, SKV], bf16, name=f"valTt{k}", tag=f"valTt{k}") for k in range(ND)]
            wq_t = [pp.tile([128, D], bf16, name=f"wqt{k}", tag=f"wqt{k}") for k in range(ND)]
            wk_t = [pp.tile([128, D], bf16, name=f"wkt{k}", tag=f"wkt{k}") for k in range(ND)]
            wv_t = [pp.tile([128, D], bf16, name=f"wvt{k}", tag=f"wvt{k}") for k in range(ND)]

            # ---- input DMAs: valT first (everything contracts over it) ----
            for k in range(ND):
                nc.sync.dma_start(valT_t[k][:], valT_d[k * 128:(k + 1) * 128, :])
            for k in range(ND):
                nc.scalar.dma_start(wv_t[k][:], wv_d[k * 128:(k + 1) * 128, :])
            for k in range(ND):
                nc.gpsimd.dma_start(wq_t[k][:], wq_d[k * 128:(k + 1) * 128, :])
            for k in range(ND):
                nc.gpsimd.dma_start(wk_t[k][:], wk_d[k * 128:(k + 1) * 128, :])
            for m in range(ND):
                nc.scalar.dma_start(bqt[m][:], bq_d[m * 128:(m + 1) * 128, :])
            for i in range(2):
                nc.scalar.dma_start(maskt[i][:], masks_d[i])
            for t in range(NKVT):
                ones_col = vaug[t][:].rearrange("p (h c) -> p h c", c=VA)[:, :, HD:VA]
                nc.vector.memset(ones_col, 1.0)

            probs_ring = {}

            def unit_q(m, ch):
                ps = projps.tile([128, 512], f32, name="psq", tag="proj")
                for k in range(ND):
                    nc.tensor.matmul(
                        ps[:],
                        wq_t[k][:, m * 128:(m + 1) * 128],
                        valT_t[k][:, W + ch * 512:W + (ch + 1) * 512],
                        start=(k == 0), stop=(k == ND - 1))
                nc.scalar.activation(
                    qT[m][:, ch * 512:(ch + 1) * 512], ps[:],
                    AF.Identity, bias=bqt[m][:], scale=1.0)

            K_SPANS = ((0, 512), (512, 1024), (1024, 1280))

            def unit_k(m, j):
                lo, hi = K_SPANS[j]
                ps = projps.tile([128, hi - lo], f32, name="psk", tag="proj")
                for k in range(ND):
                    nc.tensor.matmul(
                        ps[:],
                        wk_t[k][:, m * 128:(m + 1) * 128],
                        valT_t[k][:, lo:hi],
                        start=(k == 0), stop=(k == ND - 1))
                nc.vector.tensor_copy(kT[m][:, lo:hi], ps[:])

            V_SPANS = ((0, 512, 0), (512, 768, 8))

            def unit_v(t, half):
                lo, hi, h0 = V_SPANS[half]
                ps = projps.tile([128, hi - lo], f32, name="psv", tag="proj")
                for k in range(ND):
                    nc.tensor.matmul(
                        ps[:],
                        valT_t[k][:, t * 128:(t + 1) * 128],
                        wv_t[k][:, lo:hi],
                        start=(k == 0), stop=(k == ND - 1))
                nh = (hi - lo) // HD
                src = ps[:].rearrange("p (h c) -> p h c", c=HD)
                dst = vaug[t][:].rearrange("p (h c) -> p h c", c=VA)[:, h0:h0 + nh, 0:HD]
                nc.vector.tensor_copy(dst, src)

            # scores col order [kt1 kt0 kt3 kt2]; ka offsets within group g
            SC_BLOCKS = (1, 0, 3, 2)

            def unit_a(m, hh, g):
                ph = hh * 64
                ps = scps.tile([128, 1024], f32, name="pss", tag="scores")
                for blk, kto in enumerate(SC_BLOCKS):
                    ka = 2 * g + kto
                    nc.tensor.matmul(
                        ps[:, blk * 256:(blk + 1) * 256],
                        kT[m][ph:ph + 64, ka * 128:(ka + 1) * 128],
                        qT[m][ph:ph + 64, g * 256:(g + 1) * 256],
                        start=True, stop=True)
                probs = prp.tile([128, 1024], bf16, name="probs", tag="probs")
                nc.scalar.activation(probs[:], ps[:], AF.Exp)
                if g == 0:
                    nc.vector.tensor_mul(probs[:], probs[:], maskt[0][:])
                else:
                    nc.vector.tensor_mul(
                        probs[:, 128:896], probs[:, 128:896], maskt[1][:, 128:896])
                probs_ring[(m, hh, g)] = probs

            # probs col ranges per query half qs: (ka_offset, col)
            PV_SLICES = (((1, 0), (0, 256), (2, 768)),      # qs=0: kt1,kt0,kt2 @ r<128
                         ((1, 128), (3, 640), (2, 896)))    # qs=1: kt1,kt3,kt2 @ r>=128

            def unit_b(m, g):
                pv = pvps.tile([128, 2 * 2 * VA], f32, name="pspv", tag="pv")
                for qs in range(2):
                    for hh in range(2):
                        h = 2 * m + hh
                        probs = probs_ring[(m, hh, g)]
                        co = qs * 2 * VA + hh * VA
                        for n, (kto, c) in enumerate(PV_SLICES[qs]):
                            nc.tensor.matmul(
                                pv[:, co:co + VA],
                                probs[:, c:c + 128],
                                vaug[2 * g + kto][:, h * VA:(h + 1) * VA],
                                start=(n == 0), stop=(n == 2))
                for qs in range(2):
                    osb = outp.tile([128, 2 * VA], f32, name="osb", tag="outsb")
                    nc.vector.tensor_copy(osb[:], pv[:, qs * 2 * VA:(qs + 1) * 2 * VA])
                    row = (m * NQT + 2 * g + qs) * 128
                    nc.sync.dma_start(out_d[row:row + 128, :], osb[:])
                del probs_ring[(m, 0, g)]
                del probs_ring[(m, 1, g)]

            # ---------------- static emission weave ----------------
            unit_q(0, 0); unit_q(0, 1)
            unit_k(0, 0); unit_k(0, 1); unit_k(0, 2)

            vseq = [(t, half) for t in range(NKVT) for half in range(2)]
            seq = []
            seq += [("V", vseq[0]), ("V", vseq[1])]
            ai = [(0, hh, g) for g in range(NG) for hh in range(2)]
            vi = 2
            for idx, a in enumerate(ai):
                seq.append(("A", a))
                if vi < len(vseq):
                    seq.append(("V", vseq[vi])); vi += 1
            # remaining v units + qk(1) proj units, with B(0,g) gated on vaug
            # availability: B(0,g) needs vaug tiles <= 2g+3 -> v units 0..4g+7
            bpend = list(range(NG))
            pq1 = [("q", 1, 0), ("q", 1, 1), ("k", 1, 0), ("k", 1, 1), ("k", 1, 2)]
            while vi < len(vseq) or bpend or pq1:
                if bpend and vi >= 4 * bpend[0] + 8:
                    seq.append(("B", (0, bpend.pop(0))))
                elif pq1:
                    seq.append(("P", pq1.pop(0)))
                elif vi < len(vseq):
                    seq.append(("V", vseq[vi])); vi += 1
                else:
                    seq.append(("B", (0, bpend.pop(0))))
                if vi < len(vseq):
                    seq.append(("V", vseq[vi])); vi += 1
            for it in seq:
                if it[0] == "V":
                    unit_v(*it[1])
                elif it[0] == "A":
                    unit_a(*it[1])
                elif it[0] == "B":
                    unit_b(*it[1])
                else:
                    kind, mm, j = it[1]
                    if kind == "q":
                        unit_q(mm, j)
                    else:
                        unit_k(mm, j)

            for m in range(1, ND):
                p_units = []
                nm = m + 1
                if nm < ND:
                    p_units = [("q", nm, 0), ("q", nm, 1),
                               ("k", nm, 0), ("k", nm, 1), ("k", nm, 2)]
                # weave: A A A A B A A B A A B B with P units spliced between
                order = [("A", (m, 0, 0)), ("A", (m, 1, 0)),
                         ("A", (m, 0, 1)), ("A", (m, 1, 1)), ("B", (m, 0)),
                         ("A", (m, 0, 2)), ("A", (m, 1, 2)), ("B", (m, 1)),
                         ("A", (m, 0, 3)), ("A", (m, 1, 3)), ("B", (m, 2)),
                         ("B", (m, 3))]
                woven = []
                pi = 0
                for u in order:
                    woven.append(u)
                    if pi < len(p_units) and u[0] == "A":
                        woven.append(("P", p_units[pi])); pi += 1
                for u in woven:
                    if u[0] == "A":
                        unit_a(*u[1])
                    elif u[0] == "B":
                        unit_b(*u[1])
                    else:
                        kind, mm, j = u[1]
                        if kind == "q":
                            unit_q(mm, j)
                        else:
                            unit_k(mm, j)

    nc.compile()
    return nc


def _get_nc():
    if "nc" not in _CACHE:
        _CACHE["nc"] = _build_nc()
    return _CACHE["nc"]


def _install_ntff_hook():
    """Provide antenv.axon_hooks (absent in this image) so bass_utils can
    NTFF-profile under axon, using trn_agent_boot's ctypes hook builder."""
    import sys
    import types
    try:
        from antenv.axon_hooks import get_axon_ntff_profile_hook  # noqa: F401
        return
    except ImportError:
        pass
    try:
        import antenv
        from trn_agent_boot.trn_boot import _ntff_profile_via_ctypes
        hook = _ntff_profile_via_ctypes("/opt/axon/libaxon_pjrt.so")
        mod = types.ModuleType("antenv.axon_hooks")
        mod.get_axon_ntff_profile_hook = lambda: hook
        mod.set_axon_ntff_profile_hook = lambda h: None
        sys.modules["antenv.axon_hooks"] = mod
        antenv.axon_hooks = mod
    except Exception as e:  # profiling is best-effort
        print(f"ntff hook install failed: {e}")


def kernel(val, Wq, bq, Wk, bk, Wv, bv):
    from concourse.bass_utils import run_bass_kernel_spmd

    bf = ml_dtypes.bfloat16
    val = np.asarray(val, dtype=np.float32)
    Wq = np.asarray(Wq, dtype=np.float32)
    bq = np.asarray(bq, dtype=np.float32)
    Wk = np.asarray(Wk, dtype=np.float32)
    Wv = np.asarray(Wv, dtype=np.float32)
    bv = np.asarray(bv, dtype=np.float32)

    wq_s = np.ascontiguousarray((Wq * SCALE).astype(bf))
    bq_s = np.ascontiguousarray((bq * SCALE).reshape(D, 1))
    wk_c = np.ascontiguousarray(Wk.astype(bf))
    wv_c = np.ascontiguousarray(Wv.astype(bf))

    in_maps = []
    for c in range(NCORES):
        b, qd = divmod(c, 4)
        lo = qd * SQ - W
        hi = qd * SQ + SQ
        vs = val[max(lo, 0):hi, b, :]
        if lo < 0:
            vs = np.concatenate([np.zeros((-lo, D), np.float32), vs], axis=0)
        in_maps.append({
            "valT": np.ascontiguousarray(vs.T.astype(bf)),
            "wq": wq_s, "wk": wk_c, "wv": wv_c, "bq": bq_s,
            "masks": _masks_np(boundary=(qd == 0)),
        })

    nc = _get_nc()
    trace = os.environ.get("BASS_KERNEL_TRACE", "0") == "1"
    kwargs = {}
    if trace:
        _install_ntff_hook()
        kwargs = dict(trace=True, tmpdir=os.environ.get("BASS_KERNEL_TRACE_DIR") or None)
    res = run_bass_kernel_spmd(nc, in_maps, list(range(NCORES)), **kwargs)
    _CACHE["last_result"] = res

    out = np.empty((S, B, D), np.float32)
    for c in range(NCORES):
        b, qd = divmod(c, 4)
        raw = res.results[c]["out"].reshape(ND, NQT, 128, 2, VA)
        a = raw.transpose(1, 2, 0, 3, 4).reshape(SQ, H, VA)
        out[qd * SQ:(qd + 1) * SQ, b, :] = (
            a[:, :, 0:HD] / a[:, :, HD:VA]).reshape(SQ, D)
    out += bv
    return out


# revision 11
# speedup vs baseline: 1.5750x; 1.0327x over previous
"""Trainium2 Bass kernel for LocalSelfAttention (sliding-window, causal).

Problem: val (S=4096, B=2, D=768); q/k/v projections then Longformer-style
banded causal attention, window = 256 lookback (keys j in [i-256, i]).

Sharding: 8 cores = batch (2) x sequence quarters (4). Each core handles
1024 queries of one batch element and receives a 256-row key/value halo
(recomputed locally from val rows; no inter-core communication).

Math simplifications (exact up to float rounding):
  - bk dropped: per-query additive constant q.bk cancels in softmax.
  - bv added on host at the end: sum_j p_j (v0_j + bv) = (PV)/Z + bv.
  - no max-subtraction in softmax: scores ~ N(0,1), |s| < ~8, exp is safe.
  - 1/sqrt(hd) folded into Wq/bq on host.

v2: single interleaved phase. All inputs bf16 (FWL weight loads, half the
DMA). Projection psum tiles, attention score units and PV units are woven
into one emission stream so every engine (PE / ACT exp / DVE casts+masks)
stays busy and psum ring recycling latency hides under other PE work.

Scores psum column layout per (head, 256-query group) is [kt1 kt0 kt3 kt2]
(256 cols each, kt = key tile of 128; kt0/kt1 = the 256 keys before the
group, kt2/kt3 = the group's own keys). This makes the masked + dead region
one contiguous 768-col block (cols 128:896), handled by a single DVE
multiply. PV accumulates a head-pair into one [128, 260] psum bank:
cols = qs*130 + hh*65 + (64 values + 1 ones-column row-sum).
"""

import os
import numpy as np
import ml_dtypes

S, B, D = 4096, 2, 768
H, HD = 12, 64
W = 256
NCORES = 8
SQ = S // 4            # 1024 queries per core
SKV = SQ + W           # 1280 kv rows (halo)
NQT = SQ // 128        # 8 query tiles
NG = SQ // 256         # 4 query groups
ND = D // 128          # 6 feature tiles (also head pairs)
NKVT = SKV // 128      # 10 kv tiles
VA = HD + 1            # 65: per-head v width incl. ones column
SCALE = 1.0 / np.sqrt(HD).astype(np.float32)  # 0.125

_CACHE = {}


def _masks_np(boundary: bool) -> np.ndarray:
    """(2, 128, 1024) bf16 multiplicative masks for the [kt1 kt0 kt3 kt2]
    score layout. Partition = key-within-tile p; free = 4 blocks of 256
    query rows r (r in 0..255 within the group).

    Valid patterns (c = r mod 128):
      kt1 block (cols 0:256):    r<128 all-valid; r>=128 tri c<=p
      kt0 block (cols 256:512):  r<128 tri c<=p;  r>=128 dead
      kt3 block (cols 512:768):  r<128 dead;      r>=128 tri c>=p
      kt2 block (cols 768:1024): r<128 tri c>=p;  r>=128 all-valid
    Set 0 is used for group 0 (multiplied over the full 1024 cols; on
    sequence-boundary cores kt1+kt0 are entirely invalid), set 1 for
    groups 1..3 (multiplied over cols 128:896 only).
    """
    p = np.arange(128)[:, None]
    c = np.arange(128)[None, :]
    triL = (c <= p).astype(np.float32)
    triU = (c >= p).astype(np.float32)
    setB = np.ones((128, 1024), np.float32)
    setB[:, 128:256] = triL
    setB[:, 256:384] = triL
    setB[:, 384:640] = 0.0
    setB[:, 640:768] = triU
    setB[:, 768:896] = triU
    setA = setB.copy()
    if boundary:
        setA[:, 0:512] = 0.0    # kt1+kt0 keys are before row 0 -> invalid
    m = np.stack([setA, setB])
    return np.ascontiguousarray(m.astype(ml_dtypes.bfloat16))


def _build_nc():
    import concourse.bacc as bacc
    import concourse.mybir as mybir
    from concourse.tile import TileContext

    f32 = mybir.dt.float32
    bf16 = mybir.dt.bfloat16
    AF = mybir.ActivationFunctionType

    nc = bacc.Bacc(trn_type="TRN2", debug=False, num_devices=NCORES)

    valT_d = nc.dram_tensor("valT", [D, SKV], bf16, kind="ExternalInput").ap()
    wq_d = nc.dram_tensor("wq", [D, D], bf16, kind="ExternalInput").ap()
    wk_d = nc.dram_tensor("wk", [D, D], bf16, kind="ExternalInput").ap()
    wv_d = nc.dram_tensor("wv", [D, D], bf16, kind="ExternalInput").ap()
    bq_d = nc.dram_tensor("bq", [D, 1], f32, kind="ExternalInput").ap()
    masks_d = nc.dram_tensor("masks", [2, 128, 1024], bf16, kind="ExternalInput").ap()
    out_d = nc.dram_tensor("out", [ND * NQT * 128, 2 * VA], f32, kind="ExternalOutput").ap()

    with TileContext(nc) as tc:
        with tc.tile_pool(name="persist", bufs=1) as pp, \
             tc.tile_pool(name="projps", bufs=2, space="PSUM") as projps, \
             tc.tile_pool(name="scps", bufs=2, space="PSUM") as scps, \
             tc.tile_pool(name="pvps", bufs=2, space="PSUM") as pvps, \
             tc.tile_pool(name="probsp", bufs=6) as prp, \
             tc.tile_pool(name="outp", bufs=6) as outp:

            qT = [pp.tile([128, SQ], bf16, name=f"qT{m}", tag=f"qT{m}") for m in range(ND)]
            kT = [pp.tile([128, SKV], bf16, name=f"kT{m}", tag=f"kT{m}") for m in range(ND)]
            vaug = [pp.tile([128, H * VA], bf16, name=f"vaug{t}", tag=f"vaug{t}") for t in range(NKVT)]
            bqt = [pp.tile([128, 1], f32, name=f"bqt{m}", tag=f"bqt{m}") for m in range(ND)]
            maskt = [pp.tile([128, 1024], bf16, name=f"maskt{i}", tag=f"maskt{i}") for i in range(2)]
            valT_t = [pp.tile([128, SKV], bf16, name=f"valTt{k}", tag=f"valTt{k}") for k in range(ND)]
            wq_t = [pp.tile([128, D], bf16, name=f"wqt{k}", tag=f"wqt{k}") for k in range(ND)]
            wk_t = [pp.tile([128, D], bf16, name=f"wkt{k}", tag=f"wkt{k}") for k in range(ND)]
            wv_t = [pp.tile([128, D], bf16, name=f"wvt{k}", tag=f"wvt{k}") for k in range(ND)]

            # ---- input DMAs: round-robin the 3 issue queues; valT + wv
            # land first (v-proj is the first PE work), then wq, wk ----
            qs_ = (nc.sync, nc.scalar, nc.gpsimd)
            qi = 0

            def dma_in(dst, src):
                nonlocal qi
                qs_[qi % 3].dma_start(dst, src)
                qi += 1

            for k in range(ND):
                dma_in(valT_t[k][:], valT_d[k * 128:(k + 1) * 128, :])
            for k in range(ND):
                dma_in(wv_t[k][:], wv_d[k * 128:(k + 1) * 128, :])
            for k in range(ND):
                dma_in(wq_t[k][:], wq_d[k * 128:(k + 1) * 128, :])
            for k in range(ND):
                dma_in(wk_t[k][:], wk_d[k * 128:(k + 1) * 128, :])
            for m in range(ND):
                nc.scalar.dma_start(bqt[m][:], bq_d[m * 128:(m + 1) * 128, :])
            for i in range(2):
                nc.gpsimd.dma_start(maskt[i][:], masks_d[i])
            for t in range(NKVT):
                ones_col = vaug[t][:].rearrange("p (h c) -> p h c", c=VA)[:, :, HD:VA]
                nc.vector.memset(ones_col, 1.0)

            probs_ring = {}

            def unit_q(m, ch):
                ps = projps.tile([128, 512], f32, name="psq", tag="proj")
                for k in range(ND):
                    nc.tensor.matmul(
                        ps[:],
                        wq_t[k][:, m * 128:(m + 1) * 128],
                        valT_t[k][:, W + ch * 512:W + (ch + 1) * 512],
                        start=(k == 0), stop=(k == ND - 1))
                nc.scalar.activation(
                    qT[m][:, ch * 512:(ch + 1) * 512], ps[:],
                    AF.Identity, bias=bqt[m][:], scale=1.0)

            K_SPANS = ((0, 512), (512, 1024), (1024, 1280))

            def unit_k(m, j):
                lo, hi = K_SPANS[j]
                ps = projps.tile([128, hi - lo], f32, name="psk", tag="proj")
                for k in range(ND):
                    nc.tensor.matmul(
                        ps[:],
                        wk_t[k][:, m * 128:(m + 1) * 128],
                        valT_t[k][:, lo:hi],
                        start=(k == 0), stop=(k == ND - 1))
                nc.vector.tensor_copy(kT[m][:, lo:hi], ps[:])

            V_SPANS = ((0, 512, 0), (512, 768, 8))

            def unit_v(t, half):
                lo, hi, h0 = V_SPANS[half]
                ps = projps.tile([128, hi - lo], f32, name="psv", tag="proj")
                for k in range(ND):
                    nc.tensor.matmul(
                        ps[:],
                        valT_t[k][:, t * 128:(t + 1) * 128],
                        wv_t[k][:, lo:hi],
                        start=(k == 0), stop=(k == ND - 1))
                nh = (hi - lo) // HD
                src = ps[:].rearrange("p (h c) -> p h c", c=HD)
                dst = vaug[t][:].rearrange("p (h c) -> p h c", c=VA)[:, h0:h0 + nh, 0:HD]
                nc.vector.tensor_copy(dst, src)

            # scores col order [kt1 kt0 kt3 kt2]; ka offsets within group g
            SC_BLOCKS = (1, 0, 3, 2)

            def unit_a(m, hh, g):
                ph = hh * 64
                ps = scps.tile([128, 1024], f32, name="pss", tag="scores")
                for blk, kto in enumerate(SC_BLOCKS):
                    ka = 2 * g + kto
                    nc.tensor.matmul(
                        ps[:, blk * 256:(blk + 1) * 256],
                        kT[m][ph:ph + 64, ka * 128:(ka + 1) * 128],
                        qT[m][ph:ph + 64, g * 256:(g + 1) * 256],
                        start=True, stop=True)
                probs = prp.tile([128, 1024], bf16, name="probs", tag="probs")
                nc.scalar.activation(probs[:], ps[:], AF.Exp)
                if g == 0:
                    nc.vector.tensor_mul(probs[:], probs[:], maskt[0][:])
                else:
                    nc.vector.tensor_mul(
                        probs[:, 128:896], probs[:, 128:896], maskt[1][:, 128:896])
                probs_ring[(m, hh, g)] = probs

            # probs col ranges per query half qs: (ka_offset, col)
            PV_SLICES = (((1, 0), (0, 256), (2, 768)),      # qs=0: kt1,kt0,kt2 @ r<128
                         ((1, 128), (3, 640), (2, 896)))    # qs=1: kt1,kt3,kt2 @ r>=128

            def unit_b(m, g):
                pv = pvps.tile([128, 2 * 2 * VA], f32, name="pspv", tag="pv")
                for qs in range(2):
                    for hh in range(2):
                        h = 2 * m + hh
                        probs = probs_ring[(m, hh, g)]
                        co = qs * 2 * VA + hh * VA
                        for n, (kto, c) in enumerate(PV_SLICES[qs]):
                            nc.tensor.matmul(
                                pv[:, co:co + VA],
                                probs[:, c:c + 128],
                                vaug[2 * g + kto][:, h * VA:(h + 1) * VA],
                                start=(n == 0), stop=(n == 2))
                osb = outp.tile([128, 4 * VA], f32, name="osb", tag="outsb")
                nc.vector.tensor_copy(osb[:], pv[:])
                eng = nc.sync if (m + g) % 2 == 0 else nc.scalar
                for qs in range(2):
                    row = (m * NQT + 2 * g + qs) * 128
                    eng.dma_start(out_d[row:row + 128, :],
                                  osb[:, qs * 2 * VA:(qs + 1) * 2 * VA])
                del probs_ring[(m, 0, g)]
                del probs_ring[(m, 1, g)]

            # ---------------- static emission weave ----------------
            vseq = [(t, half) for t in range(NKVT) for half in range(2)]
            seq = [("V", vseq[0]), ("V", vseq[1]),
                   ("P", ("q", 0, 0)), ("V", vseq[2]), ("P", ("q", 0, 1)),
                   ("V", vseq[3]), ("P", ("k", 0, 0)), ("V", vseq[4]),
                   ("P", ("k", 0, 1)), ("V", vseq[5]), ("P", ("k", 0, 2))]
            ai = [(0, hh, g) for g in range(NG) for hh in range(2)]
            vi = 6
            for idx, a in enumerate(ai):
                seq.append(("A", a))
                if vi < len(vseq):
                    seq.append(("V", vseq[vi])); vi += 1
            # remaining v units + qk(1) proj units, with B(0,g) gated on vaug
            # availability: B(0,g) needs vaug tiles <= 2g+3 -> v units 0..4g+7
            bpend = list(range(NG))
            pq1 = [("q", 1, 0), ("q", 1, 1), ("k", 1, 0), ("k", 1, 1), ("k", 1, 2)]
            while vi < len(vseq) or bpend or pq1:
                if bpend and vi >= 4 * bpend[0] + 8:
                    seq.append(("B", (0, bpend.pop(0))))
                elif pq1:
                    seq.append(("P", pq1.pop(0)))
                elif vi < len(vseq):
                    seq.append(("V", vseq[vi])); vi += 1
                else:
                    seq.append(("B", (0, bpend.pop(0))))
                if vi < len(vseq):
                    seq.append(("V", vseq[vi])); vi += 1
            for it in seq:
                if it[0] == "V":
                    unit_v(*it[1])
                elif it[0] == "A":
                    unit_a(*it[1])
                elif it[0] == "B":
                    unit_b(*it[1])
                else:
                    kind, mm, j = it[1]
                    if kind == "q":
                        unit_q(mm, j)
                    else:
                        unit_k(mm, j)

            for m in range(1, ND):
                p_units = []
                nm = m + 1
                if nm < ND:
                    p_units = [("q", nm, 0), ("q", nm, 1),
                               ("k", nm, 0), ("k", nm, 1), ("k", nm, 2)]
                # weave: A A A A B A A B A A B B with P units spliced between
                order = [("A", (m, 0, 0)), ("A", (m, 1, 0)),
                         ("A", (m, 0, 1)), ("A", (m, 1, 1)), ("B", (m, 0)),
                         ("A", (m, 0, 2)), ("A", (m, 1, 2)), ("B", (m, 1)),
                         ("A", (m, 0, 3)), ("A", (m, 1, 3)), ("B", (m, 2)),
                         ("B", (m, 3))]
                woven = []
                pi = 0
                for u in order:
                    woven.append(u)
                    if pi < len(p_units) and u[0] == "A":
                        woven.append(("P", p_units[pi])); pi += 1
                for u in woven:
                    if u[0] == "A":
                        unit_a(*u[1])
                    elif u[0] == "B":
                        unit_b(*u[1])
                    else:
                        kind, mm, j = u[1]
                        if kind == "q":
                            unit_q(mm, j)
                        else:
                            unit_k(mm, j)

    nc.compile()
    return nc


def _get_nc():
    if "nc" not in _CACHE:
        _CACHE["nc"] = _build_nc()
    return _CACHE["nc"]


def _install_ntff_hook():
    """Provide antenv.axon_hooks (absent in this image) so bass_utils can
    NTFF-profile under axon, using trn_agent_boot's ctypes hook builder."""
    import sys
    import types
    try:
        from antenv.axon_hooks import get_axon_ntff_profile_hook  # noqa: F401
        return
    except ImportError:
        pass
    try:
        import antenv
        from trn_agent_boot.trn_boot import _ntff_profile_via_ctypes
        hook = _ntff_profile_via_ctypes("/opt/axon/libaxon_pjrt.so")
        mod = types.ModuleType("antenv.axon_hooks")
        mod.get_axon_ntff_profile_hook = lambda: hook
        mod.set_axon_ntff_profile_hook = lambda h: None
        sys.modules["antenv.axon_hooks"] = mod
        antenv.axon_hooks = mod
    except Exception as e:  # profiling is best-effort
        print(f"ntff hook install failed: {e}")


def kernel(val, Wq, bq, Wk, bk, Wv, bv):
    from concourse.bass_utils import run_bass_kernel_spmd

    bf = ml_dtypes.bfloat16
    val = np.asarray(val, dtype=np.float32)
    Wq = np.asarray(Wq, dtype=np.float32)
    bq = np.asarray(bq, dtype=np.float32)
    Wk = np.asarray(Wk, dtype=np.float32)
    Wv = np.asarray(Wv, dtype=np.float32)
    bv = np.asarray(bv, dtype=np.float32)

    wq_s = np.ascontiguousarray((Wq * SCALE).astype(bf))
    bq_s = np.ascontiguousarray((bq * SCALE).reshape(D, 1))
    wk_c = np.ascontiguousarray(Wk.astype(bf))
    wv_c = np.ascontiguousarray(Wv.astype(bf))

    in_maps = []
    for c in range(NCORES):
        b, qd = divmod(c, 4)
        lo = qd * SQ - W
        hi = qd * SQ + SQ
        vs = val[max(lo, 0):hi, b, :]
        if lo < 0:
            vs = np.concatenate([np.zeros((-lo, D), np.float32), vs], axis=0)
        in_maps.append({
            "valT": np.ascontiguousarray(vs.T.astype(bf)),
            "wq": wq_s, "wk": wk_c, "wv": wv_c, "bq": bq_s,
            "masks": _masks_np(boundary=(qd == 0)),
        })

    nc = _get_nc()
    trace = os.environ.get("BASS_KERNEL_TRACE", "0") == "1"
    kwargs = {}
    if trace:
        _install_ntff_hook()
        kwargs = dict(trace=True, tmpdir=os.environ.get("BASS_KERNEL_TRACE_DIR") or None)
    res = run_bass_kernel_spmd(nc, in_maps, list(range(NCORES)), **kwargs)
    _CACHE["last_result"] = res

    out = np.empty((S, B, D), np.float32)
    for c in range(NCORES):
        b, qd = divmod(c, 4)
        raw = res.results[c]["out"].reshape(ND, NQT, 128, 2, VA)
        a = raw.transpose(1, 2, 0, 3, 4).reshape(SQ, H, VA)
        out[qd * SQ:(qd + 1) * SQ, b, :] = (
            a[:, :, 0:HD] / a[:, :, HD:VA]).reshape(SQ, D)
    out += bv
    return out
